# revision 45
# baseline (speedup 1.0000x reference)
"""Trainium2 Bass kernel for the unit-commitment custom loss.

Strategy (8 NeuronCores, SPMD):
  - Generators are sorted by min_uptime U (copy-U) and independently by
    min_downtime D (copy-D); rows are dealt to cores so every core has an
    IDENTICAL per-U-value group layout (32-row aligned, padded with zero
    dummy rows).  The windowed min-up/down-time violation sums are then
    computable with one prefix-scan plus a shifted subtract per
    (tile, U-run) instead of 14 lag passes.
  - Per scenario block the binary commitment series is laid out as
    [init | s_0..s_95 | 0 x 7] (104 cols) so windows never cross blocks.
  - viol_up = sum_g (U-1)*S0 - V_U + early;  V_U = sum sw_on * W,
    W[c] = cs[c+U-1]-cs[c] (window sum of s), restricted to t <= T-U;
    S0 = sum sw_on restricted.  viol_dn = sum_g V_D + early.
  - BCE uses t ln p + (1-t) ln(1-p) = ln|p - (1-t)|, computed as
    0.5*ln((p-t')^2): one 2x-mode fp16 subtract + Square + Ln(accum).
  - seg_prod / profiled_generation / rates / curtailment are reduced on
    the Tensor engine: host-transposed [bt, cols] tiles matmul'd with a
    ones vector into PSUM accumulation chains.
  - dtypes: binaries + seg_prod fp8(e4m3) (exact for 0/1), probabilities
    fp16, all accumulation f32.
"""

import numpy as np

B, G, T, K, P, S = 16, 4000, 96, 4, 500, 200
M = 8                 # cores
BS = B // M           # scenarios per core for B-sharded tensors
GC = G // M           # 500 real generators per core
BCR = 512             # padded BCE rows per core (dense, 500 real)
BL = 1 + T + 7        # 104: init | s_0..s_95 | 7 zero pad
WID = B * BL          # 1664 (one sorted copy)
WID2 = 2 * WID        # 3328 (U copy | D copy)
PT_W = 2 * B * T      # 3072: p | t' packed
NBT = B * T           # 1536 rows of transposed seg
SEG_W = GC * K        # 2000 seg columns per core (rank-major, dense)
VROW_W = SEG_W + P + 2 * S + 1   # seg | pg | cr | dr | curt
VIOLATIONS_PENALTY = 1000.0
POWER_BALANCE_PENALTY = 5000.0

# out_g column map (violation copy rows)
C_SWON, C_VU, C_TLU, C_VD = 0, 1, 2, 3
C_A0 = 4              # cols 4..12: A'(r)=sum_b cs9[9b+r], r=0..8
NCOL = 16

_CACHE = {}


def _legal_pieces(a, b):
    """Split [a,b) (32-aligned, within a 128-row tile) into hardware-legal
    partition ranges: start 0 (<=128), 32 (<=32), 64 (<=64), 96 (<=32)."""
    out = []
    while a < b:
        if a == 0:
            e = b
        elif a == 32:
            e = min(b, 64)
        elif a == 64:
            e = min(b, 128)
        elif a == 96:
            e = b
        else:
            raise AssertionError(f"unaligned start {a}")
        out.append((a, e))
        a = e
    return out


def _tiles_of(gpc):
    out = []
    r = 0
    while r < gpc:
        n = min(128, gpc - r)
        out.append((r, n))
        r += n
    return out


def _build_nc(mu, md, gpc):
    import concourse.bacc as bacc
    import concourse.tile as tile
    import concourse.mybir as mybir

    f32 = mybir.dt.float32
    f16 = mybir.dt.float16
    f8 = mybir.dt.float8e4
    alu = mybir.AluOpType
    AX = mybir.AxisListType
    ACT = mybir.ActivationFunctionType

    nc = bacc.Bacc("TRN2", target_bir_lowering=False, debug=False, num_devices=M)

    s2_d = nc.dram_tensor("s2", [gpc, WID2], f16, kind="ExternalInput").ap()
    pt_d = nc.dram_tensor("pt", [BCR, PT_W], f16, kind="ExternalInput").ap()
    seg_d = nc.dram_tensor("seg", [NBT, SEG_W], f8, kind="ExternalInput").ap()
    pgr_d = nc.dram_tensor("pgr", [2 * T, P + 2 * S + 1], f16, kind="ExternalInput").ap()
    sto_d = nc.dram_tensor("sto", [S, 4 * BS * T], f16, kind="ExternalInput").ap()
    outg_d = nc.dram_tensor("out_g", [gpc, NCOL], f32, kind="ExternalOutput").ap()
    outb_d = nc.dram_tensor("out_b", [BCR, 4], f32, kind="ExternalOutput").ap()
    outs_d = nc.dram_tensor("out_s", [S, 4], f32, kind="ExternalOutput").ap()
    outv_d = nc.dram_tensor("out_v", [1, VROW_W], f32, kind="ExternalOutput").ap()

    tiles = _tiles_of(gpc)

    def runs_of(mdict):
        """[(tile_idx, a, b, v)] with [a,b) legal partition pieces."""
        pos = _positions(mdict, gpc)
        runs = []
        for v in sorted(mdict):
            n = mdict[v]
            if v >= 2 and v <= T and n > 0:
                r0, r1 = pos[v], pos[v] + n
                assert r1 <= gpc
                for ti, (t0, tn) in enumerate(tiles):
                    a, b = max(r0, t0), min(r1, t0 + tn)
                    if a < b:
                        for (pa, pb) in _legal_pieces(a - t0, b - t0):
                            runs.append((ti, pa, pb, v))
        return runs

    uruns = runs_of(mu)
    druns = runs_of(md)

    # seg psum chains: SEG_W split into <=512 column chunks
    segch = []
    cc = 0
    while cc < SEG_W:
        segch.append((cc, min(512, SEG_W - cc)))
        cc += 512

    with tile.TileContext(nc) as tc:
        with (
            tc.tile_pool(name="const", bufs=1) as cpool,
            tc.tile_pool(name="inp", bufs=3) as inp,
            tc.tile_pool(name="work", bufs=3) as work,
            tc.tile_pool(name="bwork", bufs=2) as bwork,
            tc.tile_pool(name="segp", bufs=3) as segp,
            tc.tile_pool(name="small", bufs=2) as smallp,
            tc.psum_pool(name="ps", bufs=1) as psp,
        ):
            ones8 = cpool.tile([128, 1], f8, tag="ones8")
            nc.vector.memset(ones8[:], 1.0)
            ones16 = cpool.tile([128, 1], f16, tag="ones16")
            nc.vector.memset(ones16[:], 1.0)
            vrow = cpool.tile([1, VROW_W], f32, tag="vrow")

            ps_seg = [
                psp.tile([1, w], f32, tag=f"ps_seg{i}", name=f"ps_seg{i}")
                for i, (c0, w) in enumerate(segch)
            ]
            ps_pg = psp.tile([1, P], f32, tag="ps_pg")
            ps_rt = psp.tile([1, 2 * S + 1], f32, tag="ps_rt")

            # ---------------- pg / rates / storage BCE (early: engines
            # are otherwise idle while the first s2 tile loads) ----------
            pgr_tiles = []
            for ht in range(2):
                pt_t = smallp.tile([T, P + 2 * S + 1], f16, tag="pgr",
                                   name=f"pgr{ht}")
                nc.scalar.dma_start(pt_t[:], pgr_d[ht * T:(ht + 1) * T, :])
                pgr_tiles.append(pt_t)
            for ht in range(2):
                nc.tensor.matmul(ps_pg[:], ones16[0:T, :],
                                 pgr_tiles[ht][:, 0:P],
                                 start=(ht == 0), stop=(ht == 1))
                nc.tensor.matmul(ps_rt[:], ones16[0:T, :],
                                 pgr_tiles[ht][:, P:P + 2 * S + 1],
                                 start=(ht == 0), stop=(ht == 1))
            nc.scalar.activation(vrow[:, SEG_W:SEG_W + P], ps_pg[:], ACT.Copy)
            nc.scalar.activation(vrow[:, SEG_W + P:VROW_W], ps_rt[:], ACT.Copy)

            # storage DMAs early (scalar queue); compute emitted after
            # tile 0 so DVE doesn't stall waiting for them at startup
            SR = S // 2
            sto_tiles = []
            for st in range(2):
                sto = smallp.tile([SR, 4 * BS * T], f16, tag="sto",
                                  name=f"sto{st}")
                nc.scalar.dma_start(sto[:], sto_d[st * SR:(st + 1) * SR, :])
                sto_tiles.append(sto)

            # ---------------- violation copies ----------------
            for ti, (g0, gn) in enumerate(tiles):
                # critical-path loads first on the sync queue (split halves
                # so first-half compute can start sooner), then this tile's
                # slice of the seg stream
                s2 = inp.tile([gn, WID2], f16, tag="s2", name=f"s2_{ti}")
                nc.sync.dma_start(s2[:, 0:WID], s2_d[g0:g0 + gn, 0:WID])
                nc.sync.dma_start(s2[:, WID:WID2], s2_d[g0:g0 + gn, WID:WID2])
                pt = None
                if ti < BCR // 128:
                    pt = inp.tile([128, PT_W], f16, tag="pt", name=f"pt{ti}")
                    nc.sync.dma_start(pt[:], pt_d[ti * 128:(ti + 1) * 128, :])
                nseg = len(tiles)
                k0, k1 = (12 * ti) // nseg, (12 * (ti + 1)) // nseg
                seg_local = []
                for kt in range(k0, k1):
                    st = segp.tile([128, SEG_W], f8, tag="seg",
                                   name=f"seg{kt}")
                    nc.sync.dma_start(st[:],
                                      seg_d[kt * 128:(kt + 1) * 128, :])
                    seg_local.append((kt, st))

                A2 = work.tile([gn, WID2], f16, tag="A2", name=f"A2_{ti}")
                A4 = work.tile([gn, WID2], f16, tag="A4", name=f"A4_{ti}")
                nsw = work.tile([gn, WID2], f16, tag="nsw", name=f"nsw{ti}")
                W = work.tile([gn, WID2], f16, tag="W", name=f"W{ti}")
                Pp = work.tile([gn, WID2], f16, tag="P", name=f"P{ti}")
                WS = work.tile([gn, WID], f16, tag="WS", name=f"WS{ti}")
                p9 = work.tile([gn, B * 9], f16, tag="p9", name=f"p9_{ti}")
                cs9 = work.tile([gn, B * 9], f16, tag="cs9", name=f"cs9_{ti}")
                outg = work.tile([gn, NCOL], f32, tag="outg", name=f"outg{ti}")
                nc.gpsimd.memset(outg[:], 0.0)

                # A2[c] = s[c]+s[c+1], A4[c] = s[c]+..+s[c+3]; emit per side
                # only when some run in this tile needs them.
                for side, runs, base in ((0, uruns, 0), (1, druns, WID)):
                    ws = {v - 1 for (rt, a, b, v) in runs if rt == ti}
                    hi = min(WID2 - 1, base + WID + 8)
                    # D-side chains run on the otherwise-idle GPSIMD (slower
                    # per-op but fully hidden: D-runs start ~half a tile
                    # after the D half of s2 lands)
                    eng = nc.vector if side == 0 else nc.gpsimd
                    if ws & {2, 3, 6, 7} or ws & {4, 5}:
                        eng.tensor_add(
                            A2[:, base:hi], s2[:, base:hi],
                            s2[:, base + 1:hi + 1])
                    if ws & {4, 5, 6, 7}:
                        eng.tensor_add(
                            A4[:, base:hi - 2], A2[:, base:hi - 2],
                            A2[:, base + 2:hi])

                # in-block restricted switch events: accum covers exactly
                # the real positions (no head-col garbage to correct)
                s2u = s2[:, 0:WID].rearrange("g (bb c) -> g bb c", bb=B)
                s2d = s2[:, WID:WID2].rearrange("g (bb c) -> g bb c", bb=B)
                nswu = nsw[:, 0:WID].rearrange("g (bb c) -> g bb c", bb=B)
                nswd = nsw[:, WID:WID2].rearrange("g (bb c) -> g bb c", bb=B)
                nc.vector.scalar_tensor_tensor(
                    out=nswu[:, :, 1:T + 1], in0=s2u[:, :, 0:T], scalar=1.0,
                    in1=s2u[:, :, 1:T + 1], op0=alu.subtract, op1=alu.mult,
                    accum_out=outg[:, C_SWON:C_SWON + 1])
                nc.vector.scalar_tensor_tensor(
                    out=nswd[:, :, 1:T + 1], in0=s2d[:, :, 1:T + 1],
                    scalar=1.0, in1=s2d[:, :, 0:T],
                    op0=alu.subtract, op1=alu.mult)

                for side, runs, base in ((0, uruns, 0), (1, druns, WID)):
                    for (rt, a, b, v) in runs:
                        if rt != ti:
                            continue
                        w = v - 1              # window length >= 1
                        nvalid = T + 1 - v     # t in [0, T-v]
                        L = WID - 8
                        # window-sum at position c is A_w[c+1]; direct views
                        # read Wt at in-block offset 2 (=c+1), composed W
                        # already holds window(c) at col c (offset 1).
                        if w == 1:
                            Wt, wib = s2, 2
                        elif w == 2:
                            Wt, wib = A2, 2
                        elif w == 4:
                            Wt, wib = A4, 2
                        else:
                            if w == 3:
                                nc.vector.tensor_add(
                                    W[a:b, base:base + L],
                                    A2[a:b, base + 1:base + 1 + L],
                                    s2[a:b, base + 3:base + 3 + L])
                            elif w == 5:
                                nc.vector.tensor_add(
                                    W[a:b, base:base + L],
                                    A4[a:b, base + 1:base + 1 + L],
                                    s2[a:b, base + 5:base + 5 + L])
                            elif w == 6:
                                nc.vector.tensor_add(
                                    W[a:b, base:base + L],
                                    A4[a:b, base + 1:base + 1 + L],
                                    A2[a:b, base + 5:base + 5 + L])
                            elif w == 7:
                                nc.vector.tensor_add(
                                    W[a:b, base:base + L],
                                    A4[a:b, base + 1:base + 1 + L],
                                    A2[a:b, base + 5:base + 5 + L])
                                nc.vector.tensor_add(
                                    W[a:b, base:base + L],
                                    W[a:b, base:base + L],
                                    s2[a:b, base + 7:base + 7 + L])
                            else:
                                raise AssertionError(w)
                            Wt, wib = W, 1
                        nswv = nsw[a:b, base:base + WID].rearrange(
                            "g (bb c) -> g bb c", bb=B)[:, :, 1:1 + nvalid]
                        Wv = Wt[a:b, base:base + WID].rearrange(
                            "g (bb c) -> g bb c", bb=B)[:, :, wib:wib + nvalid]
                        Pv = Pp[a:b, base:base + WID].rearrange(
                            "g (bb c) -> g bb c", bb=B)[:, :, 1:1 + nvalid]
                        nc.vector.tensor_mul(Pv, nswv, Wv)
                        WSv = WS[a:b, 0:B * nvalid].rearrange(
                            "g (bb c) -> g bb c", bb=B)
                        col = C_VU if side == 0 else C_VD
                        nc.scalar.activation(
                            WSv, Pv, ACT.Copy,
                            accum_out=outg[a:b, col:col + 1])
                        if side == 0:
                            tl = nsw[a:b, 0:WID].rearrange(
                                "g (bb c) -> g bb c", bb=B)[:, :, T + 2 - v:T + 1]
                            tsc = WS[a:b, 0:B * (v - 1)].rearrange(
                                "g (bb c) -> g bb c", bb=B)
                            nc.scalar.activation(
                                tsc, tl, ACT.Copy,
                                accum_out=outg[a:b, C_TLU:C_TLU + 1])

                # PFB: prefix over each block's first 9 cols (init + s0..s7)
                s9v = s2[:, 0:WID].rearrange(
                    "g (bb c) -> g bb c", c=BL)[:, :, 0:9]
                nc.scalar.activation(
                    p9[:].rearrange("g (bb c) -> g bb c", c=9), s9v, ACT.Copy)
                nc.vector.tensor_tensor_scan(
                    cs9[:], p9[:], p9[:], 0.0, alu.add, alu.max)
                nc.vector.tensor_reduce(
                    outg[:, C_A0:C_A0 + 9],
                    cs9[:].rearrange("g (bb c) -> g c bb", c=9),
                    axis=AX.X, op=alu.add)

                nc.sync.dma_start(outg_d[g0:g0 + gn, :], outg[:])

                # interleave seg matmul chains across violation tiles
                for kt, st in seg_local:
                    for i, (c0, wdt) in enumerate(segch):
                        nc.tensor.matmul(
                            ps_seg[i][:], ones8[:],
                            st[:, c0:c0 + wdt],
                            start=(kt == 0), stop=(kt == 11))

                # storage BCE compute after tile 0 (overlaps tile 1 load)
                if ti == 0:
                    for st in range(2):
                        sto = sto_tiles[st]
                        dsto = smallp.tile([SR, 2 * BS * T], f16, tag="dsto",
                                           name=f"dsto{st}")
                        ssto = smallp.tile([SR, 2 * BS * T], f16, tag="ssto",
                                           name=f"ssto{st}")
                        souts = smallp.tile([SR, 4], f32, tag="souts",
                                            name=f"souts{st}")
                        nc.gpsimd.memset(souts[:], 0.0)
                        w = BS * T
                        nc.vector.tensor_sub(dsto[:, 0:w], sto[:, 0:w],
                                             sto[:, w:2 * w])
                        nc.vector.tensor_sub(dsto[:, w:2 * w],
                                             sto[:, 2 * w:3 * w],
                                             sto[:, 3 * w:4 * w])
                        nc.scalar.activation(ssto[:], dsto[:], ACT.Square)
                        nc.scalar.activation(dsto[:], ssto[:], ACT.Ln,
                                             accum_out=souts[:, 0:1])
                        nc.sync.dma_start(outs_d[st * SR:(st + 1) * SR, :],
                                          souts[:])

                # interleave one BCE tile per violation tile
                if pt is not None:
                    bi, r0 = ti, ti * 128
                    dt_ = bwork.tile([128, B * T], f16, tag="d", name=f"d{bi}")
                    sq = bwork.tile([128, B * T], f16, tag="sq", name=f"sq{bi}")
                    outb = bwork.tile([128, 4], f32, tag="outb",
                                      name=f"outb{bi}")
                    nc.gpsimd.memset(outb[:], 0.0)
                    nc.vector.tensor_sub(dt_[:], pt[:, 0:B * T],
                                         pt[:, B * T:PT_W])
                    nc.scalar.activation(sq[:], dt_[:], ACT.Square)
                    nc.scalar.activation(dt_[:], sq[:], ACT.Ln,
                                         accum_out=outb[:, 0:1])
                    nc.sync.dma_start(outb_d[r0:r0 + 128, :], outb[:])

            # seg chain results -> vrow -> out (after last seg matmul)
            for i, (c0, wdt) in enumerate(segch):
                nc.scalar.activation(vrow[:, c0:c0 + wdt], ps_seg[i][:], ACT.Copy)
            nc.sync.dma_start(outv_d[:, :], vrow[:])

    nc.compile()
    return nc


def _get_nc(mu_t, md_t, gpc):
    key = (mu_t, md_t, gpc)
    if key not in _CACHE:
        _CACHE[key] = _build_nc(dict(mu_t), dict(md_t), gpc)
    return _CACHE[key]


def _plan(vals):
    """32-aligned identical per-core group layout.
    Returns (m: value->rows_per_core (aligned), order: sorted g indices)."""
    order = np.argsort(vals, kind="stable")
    uniq, counts = np.unique(vals, return_counts=True)
    m = {}
    for v, n in zip(uniq.tolist(), counts.tolist()):
        mv = -(-n // M)            # ceil
        m[int(v)] = -(-mv // 32) * 32
    return m, order, uniq.tolist(), counts.tolist()


def _positions(m, gpc):
    """Choose a group ordering minimizing legal-piece splits.
    Returns pos: v -> starting row.  Deterministic (shared by host prep
    and program builder)."""
    vs = [v for v in sorted(m) if m[v] > 0]
    tiles = _tiles_of(gpc)

    def pieces_for(order):
        off = 0
        pos = {}
        for v in order:
            pos[v] = off
            off += m[v]
        n = 0
        for v in order:
            if v < 2 or v > T:
                continue
            r0, r1 = pos[v], pos[v] + m[v]
            for (t0, tn) in tiles:
                a, b = max(r0, t0), min(r1, t0 + tn)
                if a < b:
                    n += len(_legal_pieces(a - t0, b - t0))
        return n, pos

    cands = [
        list(vs),
        sorted(vs, key=lambda v: (m[v], v)),
        sorted(vs, key=lambda v: (-m[v], v)),
        sorted(vs, key=lambda v: (m[v] != 64, m[v], v)),
        sorted(vs, key=lambda v: (m[v] != 64, -m[v], v)),
    ]
    best = None
    for order in cands:
        n, pos = pieces_for(order)
        if best is None or n < best[0]:
            best = (n, pos)
    return best[1]


def _pack(m):
    """Place groups at tile offsets so each (tile, group) intersection is a
    single legal partition range.  Returns (pos: v->row, gpc).
    Groups sized {32,64,96} pack into 128-row tiles; larger groups fall
    back to sequential placement (correct, just more run pieces)."""
    sizes = {v: s for v, s in m.items() if s > 0}
    if any(s > 96 for s in sizes.values()):
        pos = {}
        off = 0
        for v in sorted(sizes):
            pos[v] = off
            off += sizes[v]
        return pos, -(-off // 128) * 128
    pos = {}
    t = 0
    slots32 = []
    slots64 = []
    for v in sorted([v for v, s in sizes.items() if s == 96]):
        pos[v] = 128 * t
        slots32.append(128 * t + 96)
        t += 1
    g64 = sorted([v for v, s in sizes.items() if s == 64])
    for i, v in enumerate(g64):
        if i % 2 == 0:
            pos[v] = 128 * t
            if i == len(g64) - 1:
                slots64.append(128 * t + 64)
            t += 1
        else:
            pos[v] = 128 * (t - 1) + 64
    for v in sorted([v for v, s in sizes.items() if s == 32]):
        if slots32:
            pos[v] = slots32.pop()
        elif slots64:
            s = slots64.pop()
            pos[v] = s
            slots32.append(s + 32)
        else:
            pos[v] = 128 * t
            slots32.append(128 * t + 32)
            slots64.append(128 * t + 64)
            t += 1
    return pos, 128 * t


def _gsel_of(m, order, uniq, counts, gpc):
    """Balanced assignment: per value, cores with the smallest running
    totals take the remainder rows, keeping per-core real counts equal."""
    gsel = -np.ones((M, gpc), dtype=np.int64)
    pos = _positions(m, gpc)
    totals = np.zeros(M, dtype=np.int64)
    start = 0
    for v, n in zip(uniq, counts):
        arr = order[start:start + n]
        start += n
        base = n // M
        rem = n - base * M
        cnt = np.full(M, base, dtype=np.int64)
        if rem:
            pick = np.argsort(totals, kind="stable")[:rem]
            cnt[pick] += 1
        a = 0
        for c in range(M):
            take = arr[a:a + cnt[c]]
            a += cnt[c]
            gsel[c, pos[v]:pos[v] + len(take)] = take
            totals[c] += cnt[c]
    return gsel


def _prep(inputs):
    import concourse.mybir as mybir
    f16 = np.float16
    f8 = mybir.dt.np(mybir.dt.float8e4)

    U = np.maximum(np.asarray(inputs["min_uptimes"], dtype=np.int64), 0)
    D = np.maximum(np.asarray(inputs["min_downtimes"], dtype=np.int64), 0)
    mu, ordU, unU, cnU = _plan(U)
    md, ordD, unD, cnD = _plan(D)
    gpc = max(sum(mu.values()), sum(md.values()))
    gpc = -(-gpc // 64) * 64
    for mm in (mu, md):
        used = sum(mm.values())
        if used < gpc:
            mm[0] = mm.get(0, 0) + (gpc - used)
    gselU = _gsel_of(mu, ordU, unU, cnU, gpc)
    gselD = _gsel_of(md, ordD, unD, cnD, gpc)

    s_full = np.asarray(inputs["thermal_on_rounded"], dtype=np.float32)
    ic = np.asarray(inputs["initial_commitment"], dtype=np.float32)
    p_full = np.asarray(inputs["thermal_on"], dtype=np.float32)
    t_full = np.asarray(inputs["tgt_thermal_commitment"], dtype=np.float32)
    sp_full = np.asarray(inputs["seg_prod"], dtype=np.float32)
    pg_full = np.asarray(inputs["profiled_generation"], dtype=np.float32)
    cr_full = np.asarray(inputs["charge_rate"], dtype=np.float32)
    dr_full = np.asarray(inputs["discharge_rate"], dtype=np.float32)
    curt = np.asarray(inputs["curtailment"], dtype=np.float32)
    chp = np.asarray(inputs["is_charging"], dtype=np.float32)
    cht = np.asarray(inputs["tgt_is_charging"], dtype=np.float32)
    dsp = np.asarray(inputs["is_discharging"], dtype=np.float32)
    dst = np.asarray(inputs["tgt_is_discharging"], dtype=np.float32)

    def spad_for(gsel):
        out = np.zeros((gpc, B, BL), dtype=np.float32)
        real = gsel >= 0
        gs = gsel[real]
        out[real, :, 0] = ic[:, gs].T
        out[real, :, 1:1 + T] = s_full[:, gs, :].transpose(1, 0, 2)
        return out.reshape(gpc, WID)

    in_maps = []
    ranks = []   # per-core dense generator order (for seg cols + bce rows)
    for c in range(M):
        gU, gD = gselU[c], gselD[c]
        s2 = np.concatenate([spad_for(gU), spad_for(gD)], axis=1)

        gs = gU[gU >= 0]         # dense rank order, len == GC
        assert len(gs) == GC
        ranks.append(gs)

        ptm = np.empty((BCR, PT_W), dtype=np.float32)
        ptm[:, 0:B * T] = 0.5
        ptm[:, B * T:] = -0.5
        ptm[:GC, 0:B * T] = p_full[:, gs, :].transpose(1, 0, 2).reshape(GC, B * T)
        ptm[:GC, B * T:] = (1.0 - t_full[:, gs, :]).transpose(1, 0, 2).reshape(GC, B * T)

        seg = sp_full[:, gs, :, :].transpose(0, 2, 1, 3).reshape(NBT, SEG_W)

        bsl = slice(BS * c, BS * (c + 1))
        pgr = np.zeros((2 * T, P + 2 * S + 1), dtype=np.float32)
        pgr[:, 0:P] = pg_full[bsl].transpose(0, 2, 1).reshape(2 * T, P)
        pgr[:, P:P + S] = cr_full[bsl].transpose(0, 2, 1).reshape(2 * T, S)
        pgr[:, P + S:P + 2 * S] = dr_full[bsl].transpose(0, 2, 1).reshape(2 * T, S)
        pgr[:, P + 2 * S] = curt[bsl].reshape(2 * T)

        w = BS * T
        sto = np.empty((S, 4 * w), dtype=np.float32)
        sto[:, 0:w] = chp[bsl].transpose(1, 0, 2).reshape(S, w)
        sto[:, w:2 * w] = (1.0 - cht[bsl]).transpose(1, 0, 2).reshape(S, w)
        sto[:, 2 * w:3 * w] = dsp[bsl].transpose(1, 0, 2).reshape(S, w)
        sto[:, 3 * w:4 * w] = (1.0 - dst[bsl]).transpose(1, 0, 2).reshape(S, w)

        in_maps.append({
            "s2": s2.astype(f16),
            "pt": ptm.astype(f16),
            "seg": np.ascontiguousarray(seg).astype(f8),
            "pgr": pgr.astype(f16),
            "sto": sto.astype(f16),
        })
    meta = (mu, md, gselU, gselD, gpc, ranks)
    return in_maps, meta


def _combine(outs, inputs, meta):
    mu, md, gselU, gselD, gpc, ranks = meta
    U_all = np.maximum(np.asarray(inputs["min_uptimes"]).astype(np.int64), 0)
    D_all = np.maximum(np.asarray(inputs["min_downtimes"]).astype(np.int64), 0)
    stat = np.asarray(inputs["initial_status"]).astype(np.int64)
    ic = np.asarray(inputs["initial_commitment"], dtype=np.float64)
    suc = np.asarray(inputs["start_up_costs"], dtype=np.float64)
    segc = np.asarray(inputs["segment_cost"], dtype=np.float64)[:, 0, :]
    puc = np.asarray(inputs["profiled_units_cost"], dtype=np.float64)
    ccost = np.asarray(inputs["charge_costs"], dtype=np.float64)
    dcost = np.asarray(inputs["discharge_costs"], dtype=np.float64)
    init_sum1 = ic[1:, :].sum(axis=0)

    viol = 0.0
    ed = 0.0
    bce_th = 0.0
    bce_sto = 0.0
    PFB_all = np.zeros((G, 9))

    for c in range(M):
        og = np.asarray(outs[c]["out_g"], dtype=np.float64)
        ob = np.asarray(outs[c]["out_b"], dtype=np.float64)
        ov = np.asarray(outs[c]["out_v"], dtype=np.float64)[0]
        os_ = np.asarray(outs[c]["out_s"], dtype=np.float64)

        gU, gD = gselU[c], gselD[c]
        rU = gU >= 0
        gs = gU[rU]
        Ug = U_all[gs]

        swon = -og[rU, C_SWON]
        tail = -og[rU, C_TLU]
        S0 = swon - tail
        V_U = -og[rU, C_VU]
        up = np.where((Ug >= 1) & (Ug <= T), (Ug - 1) * S0 - V_U, 0.0)
        viol += up.sum()

        rD = gD >= 0
        gsd = gD[rD]
        Dg = D_all[gsd]
        V_D = -og[rD, C_VD]
        viol += np.where((Dg >= 1) & (Dg <= T), V_D, 0.0).sum()

        PFB_all[gs, 1:9] = (og[rU, C_A0 + 1:C_A0 + 9]
                            - og[rU, C_A0:C_A0 + 1])

        ed += (suc[gs] * swon).sum()
        segv = ov[0:SEG_W].reshape(GC, K)
        ed += (segc[ranks[c]] * segv).sum()
        ed += (puc * ov[SEG_W:SEG_W + P]).sum()
        ed += (ccost * ov[SEG_W + P:SEG_W + P + S]).sum()
        ed += (dcost * ov[SEG_W + P + S:SEG_W + P + 2 * S]).sum()
        ed += POWER_BALANCE_PENALTY * ov[SEG_W + P + 2 * S]

        bce_th += ob[:, 0].sum()
        bce_sto += os_[:, 0].sum()

    rem_up = np.maximum(U_all - np.maximum(stat, 0), 0)
    rem_dn = np.maximum(D_all - np.maximum(-stat, 0), 0)
    g_idx = np.arange(G)
    viol += (B * rem_up - PFB_all[g_idx, rem_up]).sum()
    viol += PFB_all[g_idx, rem_dn].sum()

    sup = -(0.5 * bce_th) / float(B * G * T) - (0.5 * bce_sto) / float(B * S * T)
    total = ed + sup + VIOLATIONS_PENALTY * viol
    return np.float32(total)


def kernel(**inputs):
    from concourse.bass_utils import run_bass_kernel_spmd

    in_maps, meta = _prep(inputs)
    mu, md, gpc = meta[0], meta[1], meta[4]
    nc = _get_nc(tuple(sorted(mu.items())), tuple(sorted(md.items())), gpc)
    res = run_bass_kernel_spmd(nc, in_maps, core_ids=list(range(M)))
    return _combine(res.results, inputs, meta)


# revision 46
# speedup vs baseline: 1.1027x; 1.1027x over previous
"""Trainium2 Bass kernel for the unit-commitment custom loss.

Strategy (8 NeuronCores, SPMD):
  - Generators are sorted by min_uptime U (copy-U) and independently by
    min_downtime D (copy-D); rows are dealt to cores so every core has an
    IDENTICAL per-U-value group layout (32-row aligned, padded with zero
    dummy rows).  The windowed min-up/down-time violation sums are then
    computable with one prefix-scan plus a shifted subtract per
    (tile, U-run) instead of 14 lag passes.
  - Per scenario block the binary commitment series is laid out as
    [init | s_0..s_95 | 0 x 7] (104 cols) so windows never cross blocks.
  - viol_up = sum_g (U-1)*S0 - V_U + early;  V_U = sum sw_on * W,
    W[c] = cs[c+U-1]-cs[c] (window sum of s), restricted to t <= T-U;
    S0 = sum sw_on restricted.  viol_dn = sum_g V_D + early.
  - BCE uses t ln p + (1-t) ln(1-p) = ln|p - (1-t)|, computed as
    0.5*ln((p-t')^2): one 2x-mode fp16 subtract + Square + Ln(accum).
  - seg_prod / profiled_generation / rates / curtailment are reduced on
    the Tensor engine: host-transposed [bt, cols] tiles matmul'd with a
    ones vector into PSUM accumulation chains.
  - dtypes: binaries + seg_prod fp8(e4m3) (exact for 0/1), probabilities
    fp16, all accumulation f32.
"""

import numpy as np

B, G, T, K, P, S = 16, 4000, 96, 4, 500, 200
M = 8                 # cores
BS = B // M           # scenarios per core for B-sharded tensors
GC = G // M           # 500 real generators per core
BCR = 512             # padded BCE rows per core (dense, 500 real)
BL = 1 + T + 7        # 104: init | s_0..s_95 | 7 zero pad
WID = B * BL          # 1664 (one sorted copy)
WID2 = 2 * WID        # 3328 (U copy | D copy)
PT_W = 2 * B * T      # 3072: p | t' packed
NBT = B * T           # 1536 rows of transposed seg
SEG_W = GC * K        # 2000 seg columns per core (rank-major, dense)
VROW_W = SEG_W + P + 2 * S + 1   # seg | pg | cr | dr | curt
VIOLATIONS_PENALTY = 1000.0
POWER_BALANCE_PENALTY = 5000.0

# out_g column map (violation copy rows)
C_SWON, C_VU, C_TLU, C_VD = 0, 1, 2, 3
C_A0 = 4              # cols 4..12: A'(r)=sum_b cs9[9b+r], r=0..8
NCOL = 16

_CACHE = {}


def _legal_pieces(a, b):
    """Split [a,b) (32-aligned, within a 128-row tile) into hardware-legal
    partition ranges: start 0 (<=128), 32 (<=32), 64 (<=64), 96 (<=32)."""
    out = []
    while a < b:
        if a == 0:
            e = b
        elif a == 32:
            e = min(b, 64)
        elif a == 64:
            e = min(b, 128)
        elif a == 96:
            e = b
        else:
            raise AssertionError(f"unaligned start {a}")
        out.append((a, e))
        a = e
    return out


def _tiles_of(gpc):
    out = []
    r = 0
    while r < gpc:
        n = min(128, gpc - r)
        out.append((r, n))
        r += n
    return out


def _build_nc(mu, md, gpc):
    import concourse.bacc as bacc
    import concourse.tile as tile
    import concourse.mybir as mybir

    f32 = mybir.dt.float32
    f16 = mybir.dt.float16
    f8 = mybir.dt.float8e4
    alu = mybir.AluOpType
    AX = mybir.AxisListType
    ACT = mybir.ActivationFunctionType

    nc = bacc.Bacc("TRN2", target_bir_lowering=False, debug=False, num_devices=M)

    s2_d = nc.dram_tensor("s2", [gpc, WID2], f16, kind="ExternalInput").ap()
    pt_d = nc.dram_tensor("pt", [BCR, PT_W], f16, kind="ExternalInput").ap()
    seg_d = nc.dram_tensor("seg", [NBT, SEG_W], f8, kind="ExternalInput").ap()
    pgr_d = nc.dram_tensor("pgr", [2 * T, P + 2 * S + 1], f16, kind="ExternalInput").ap()
    sto_d = nc.dram_tensor("sto", [S, 4 * BS * T], f16, kind="ExternalInput").ap()
    outg_d = nc.dram_tensor("out_g", [gpc, NCOL], f32, kind="ExternalOutput").ap()
    outb_d = nc.dram_tensor("out_b", [BCR, 4], f32, kind="ExternalOutput").ap()
    outs_d = nc.dram_tensor("out_s", [S, 4], f32, kind="ExternalOutput").ap()
    outv_d = nc.dram_tensor("out_v", [1, VROW_W], f32, kind="ExternalOutput").ap()

    tiles = _tiles_of(gpc)

    def runs_of(mdict):
        """[(tile_idx, a, b, v)] with [a,b) legal partition pieces."""
        pos = _positions(mdict, gpc)
        runs = []
        for v in sorted(mdict):
            n = mdict[v]
            if v >= 2 and v <= T and n > 0:
                r0, r1 = pos[v], pos[v] + n
                assert r1 <= gpc
                for ti, (t0, tn) in enumerate(tiles):
                    a, b = max(r0, t0), min(r1, t0 + tn)
                    if a < b:
                        for (pa, pb) in _legal_pieces(a - t0, b - t0):
                            runs.append((ti, pa, pb, v))
        return runs

    uruns = runs_of(mu)
    druns = runs_of(md)

    # seg psum chains: SEG_W split into <=512 column chunks
    segch = []
    cc = 0
    while cc < SEG_W:
        segch.append((cc, min(512, SEG_W - cc)))
        cc += 512

    with tile.TileContext(nc) as tc:
        with (
            tc.tile_pool(name="const", bufs=1) as cpool,
            tc.tile_pool(name="inp", bufs=3) as inp,
            tc.tile_pool(name="work", bufs=3) as work,
            tc.tile_pool(name="bwork", bufs=2) as bwork,
            tc.tile_pool(name="segp", bufs=3) as segp,
            tc.tile_pool(name="small", bufs=2) as smallp,
            tc.psum_pool(name="ps", bufs=1) as psp,
        ):
            ones8 = cpool.tile([128, 1], f8, tag="ones8")
            nc.vector.memset(ones8[:], 1.0)
            ones16 = cpool.tile([128, 1], f16, tag="ones16")
            nc.vector.memset(ones16[:], 1.0)
            vrow = cpool.tile([1, VROW_W], f32, tag="vrow")

            ps_seg = [
                psp.tile([1, w], f32, tag=f"ps_seg{i}", name=f"ps_seg{i}")
                for i, (c0, w) in enumerate(segch)
            ]
            ps_pg = psp.tile([1, P], f32, tag="ps_pg")
            ps_rt = psp.tile([1, 2 * S + 1], f32, tag="ps_rt")

            # ---------------- pg / rates / storage BCE (early: engines
            # are otherwise idle while the first s2 tile loads) ----------
            pgr_tiles = []
            for ht in range(2):
                pt_t = smallp.tile([T, P + 2 * S + 1], f16, tag="pgr",
                                   name=f"pgr{ht}")
                nc.scalar.dma_start(pt_t[:], pgr_d[ht * T:(ht + 1) * T, :])
                pgr_tiles.append(pt_t)
            for ht in range(2):
                nc.tensor.matmul(ps_pg[:], ones16[0:T, :],
                                 pgr_tiles[ht][:, 0:P],
                                 start=(ht == 0), stop=(ht == 1))
                nc.tensor.matmul(ps_rt[:], ones16[0:T, :],
                                 pgr_tiles[ht][:, P:P + 2 * S + 1],
                                 start=(ht == 0), stop=(ht == 1))
            nc.scalar.activation(vrow[:, SEG_W:SEG_W + P], ps_pg[:], ACT.Copy)
            nc.scalar.activation(vrow[:, SEG_W + P:VROW_W], ps_rt[:], ACT.Copy)

            # storage DMAs early (scalar queue); compute emitted after
            # tile 0 so DVE doesn't stall waiting for them at startup
            SR = S // 2
            sto_tiles = []
            for st in range(2):
                sto = smallp.tile([SR, 4 * BS * T], f16, tag="sto",
                                  name=f"sto{st}")
                nc.scalar.dma_start(sto[:], sto_d[st * SR:(st + 1) * SR, :])
                sto_tiles.append(sto)

            # ---------------- violation copies ----------------
            for ti, (g0, gn) in enumerate(tiles):
                # critical-path loads first on the sync queue (split halves
                # so first-half compute can start sooner), then this tile's
                # slice of the seg stream
                s2 = inp.tile([gn, WID2], f16, tag="s2", name=f"s2_{ti}")
                nc.sync.dma_start(s2[:, 0:WID], s2_d[g0:g0 + gn, 0:WID])
                nc.sync.dma_start(s2[:, WID:WID2], s2_d[g0:g0 + gn, WID:WID2])
                pt = None
                if ti < BCR // 128:
                    pt = inp.tile([128, PT_W], f16, tag="pt", name=f"pt{ti}")
                    nc.sync.dma_start(pt[:], pt_d[ti * 128:(ti + 1) * 128, :])
                nseg = len(tiles)
                k0, k1 = (12 * ti) // nseg, (12 * (ti + 1)) // nseg
                seg_local = []
                for kt in range(k0, k1):
                    st = segp.tile([128, SEG_W], f8, tag="seg",
                                   name=f"seg{kt}")
                    nc.sync.dma_start(st[:],
                                      seg_d[kt * 128:(kt + 1) * 128, :])
                    seg_local.append((kt, st))

                A2 = work.tile([gn, WID2], f16, tag="A2", name=f"A2_{ti}")
                A4 = work.tile([gn, WID2], f16, tag="A4", name=f"A4_{ti}")
                nsw = work.tile([gn, WID2], f16, tag="nsw", name=f"nsw{ti}")
                W = work.tile([gn, WID2], f16, tag="W", name=f"W{ti}")
                Pp = work.tile([gn, WID2], f16, tag="P", name=f"P{ti}")
                WS = work.tile([gn, WID], f16, tag="WS", name=f"WS{ti}")
                p9 = work.tile([gn, B * 9], f16, tag="p9", name=f"p9_{ti}")
                cs9 = work.tile([gn, B * 9], f16, tag="cs9", name=f"cs9_{ti}")
                outg = work.tile([gn, NCOL], f32, tag="outg", name=f"outg{ti}")
                nc.gpsimd.memset(outg[:], 0.0)

                # A2[c] = s[c]+s[c+1], A4[c] = s[c]+..+s[c+3]; emit per side
                # only when some run in this tile needs them.
                for side, runs, base in ((0, uruns, 0), (1, druns, WID)):
                    ws = {v - 1 for (rt, a, b, v) in runs if rt == ti}
                    hi = min(WID2 - 1, base + WID + 8)
                    if ws & {2, 3, 6, 7} or ws & {4, 5}:
                        nc.vector.tensor_add(
                            A2[:, base:hi], s2[:, base:hi],
                            s2[:, base + 1:hi + 1])
                    if ws & {4, 5, 6, 7}:
                        nc.vector.tensor_add(
                            A4[:, base:hi - 2], A2[:, base:hi - 2],
                            A2[:, base + 2:hi])

                # in-block restricted switch events: accum covers exactly
                # the real positions (no head-col garbage to correct)
                s2u = s2[:, 0:WID].rearrange("g (bb c) -> g bb c", bb=B)
                s2d = s2[:, WID:WID2].rearrange("g (bb c) -> g bb c", bb=B)
                nswu = nsw[:, 0:WID].rearrange("g (bb c) -> g bb c", bb=B)
                nswd = nsw[:, WID:WID2].rearrange("g (bb c) -> g bb c", bb=B)
                nc.vector.scalar_tensor_tensor(
                    out=nswu[:, :, 1:T + 1], in0=s2u[:, :, 0:T], scalar=1.0,
                    in1=s2u[:, :, 1:T + 1], op0=alu.subtract, op1=alu.mult,
                    accum_out=outg[:, C_SWON:C_SWON + 1])
                nc.vector.scalar_tensor_tensor(
                    out=nswd[:, :, 1:T + 1], in0=s2d[:, :, 1:T + 1],
                    scalar=1.0, in1=s2d[:, :, 0:T],
                    op0=alu.subtract, op1=alu.mult)

                for side, runs, base in ((0, uruns, 0), (1, druns, WID)):
                    for (rt, a, b, v) in runs:
                        if rt != ti:
                            continue
                        w = v - 1              # window length >= 1
                        nvalid = T + 1 - v     # t in [0, T-v]
                        L = WID - 8
                        # window-sum at position c is A_w[c+1]; direct views
                        # read Wt at in-block offset 2 (=c+1), composed W
                        # already holds window(c) at col c (offset 1).
                        if w == 1:
                            Wt, wib = s2, 2
                        elif w == 2:
                            Wt, wib = A2, 2
                        elif w == 4:
                            Wt, wib = A4, 2
                        else:
                            if w == 3:
                                nc.vector.tensor_add(
                                    W[a:b, base:base + L],
                                    A2[a:b, base + 1:base + 1 + L],
                                    s2[a:b, base + 3:base + 3 + L])
                            elif w == 5:
                                nc.vector.tensor_add(
                                    W[a:b, base:base + L],
                                    A4[a:b, base + 1:base + 1 + L],
                                    s2[a:b, base + 5:base + 5 + L])
                            elif w == 6:
                                nc.vector.tensor_add(
                                    W[a:b, base:base + L],
                                    A4[a:b, base + 1:base + 1 + L],
                                    A2[a:b, base + 5:base + 5 + L])
                            elif w == 7:
                                nc.vector.tensor_add(
                                    W[a:b, base:base + L],
                                    A4[a:b, base + 1:base + 1 + L],
                                    A2[a:b, base + 5:base + 5 + L])
                                nc.vector.tensor_add(
                                    W[a:b, base:base + L],
                                    W[a:b, base:base + L],
                                    s2[a:b, base + 7:base + 7 + L])
                            else:
                                raise AssertionError(w)
                            Wt, wib = W, 1
                        nswv = nsw[a:b, base:base + WID].rearrange(
                            "g (bb c) -> g bb c", bb=B)[:, :, 1:1 + nvalid]
                        Wv = Wt[a:b, base:base + WID].rearrange(
                            "g (bb c) -> g bb c", bb=B)[:, :, wib:wib + nvalid]
                        Pv = Pp[a:b, base:base + WID].rearrange(
                            "g (bb c) -> g bb c", bb=B)[:, :, 1:1 + nvalid]
                        nc.vector.tensor_mul(Pv, nswv, Wv)
                        WSv = WS[a:b, 0:B * nvalid].rearrange(
                            "g (bb c) -> g bb c", bb=B)
                        col = C_VU if side == 0 else C_VD
                        nc.scalar.activation(
                            WSv, Pv, ACT.Copy,
                            accum_out=outg[a:b, col:col + 1])
                        if side == 0:
                            tl = nsw[a:b, 0:WID].rearrange(
                                "g (bb c) -> g bb c", bb=B)[:, :, T + 2 - v:T + 1]
                            tsc = WS[a:b, 0:B * (v - 1)].rearrange(
                                "g (bb c) -> g bb c", bb=B)
                            nc.scalar.activation(
                                tsc, tl, ACT.Copy,
                                accum_out=outg[a:b, C_TLU:C_TLU + 1])

                # PFB: prefix over each block's first 9 cols (init + s0..s7)
                s9v = s2[:, 0:WID].rearrange(
                    "g (bb c) -> g bb c", c=BL)[:, :, 0:9]
                nc.scalar.activation(
                    p9[:].rearrange("g (bb c) -> g bb c", c=9), s9v, ACT.Copy)
                nc.vector.tensor_tensor_scan(
                    cs9[:], p9[:], p9[:], 0.0, alu.add, alu.max)
                nc.vector.tensor_reduce(
                    outg[:, C_A0:C_A0 + 9],
                    cs9[:].rearrange("g (bb c) -> g c bb", c=9),
                    axis=AX.X, op=alu.add)

                nc.sync.dma_start(outg_d[g0:g0 + gn, :], outg[:])

                # interleave seg matmul chains across violation tiles
                for kt, st in seg_local:
                    for i, (c0, wdt) in enumerate(segch):
                        nc.tensor.matmul(
                            ps_seg[i][:], ones8[:],
                            st[:, c0:c0 + wdt],
                            start=(kt == 0), stop=(kt == 11))

                # storage BCE compute after tile 0 (overlaps tile 1 load)
                if ti == 0:
                    for st in range(2):
                        sto = sto_tiles[st]
                        dsto = smallp.tile([SR, 2 * BS * T], f16, tag="dsto",
                                           name=f"dsto{st}")
                        ssto = smallp.tile([SR, 2 * BS * T], f16, tag="ssto",
                                           name=f"ssto{st}")
                        souts = smallp.tile([SR, 4], f32, tag="souts",
                                            name=f"souts{st}")
                        nc.gpsimd.memset(souts[:], 0.0)
                        w = BS * T
                        nc.vector.tensor_sub(dsto[:, 0:w], sto[:, 0:w],
                                             sto[:, w:2 * w])
                        nc.vector.tensor_sub(dsto[:, w:2 * w],
                                             sto[:, 2 * w:3 * w],
                                             sto[:, 3 * w:4 * w])
                        nc.scalar.activation(ssto[:], dsto[:], ACT.Square)
                        nc.scalar.activation(dsto[:], ssto[:], ACT.Ln,
                                             accum_out=souts[:, 0:1])
                        nc.sync.dma_start(outs_d[st * SR:(st + 1) * SR, :],
                                          souts[:])

                # interleave one BCE tile per violation tile
                if pt is not None:
                    bi, r0 = ti, ti * 128
                    dt_ = bwork.tile([128, B * T], f16, tag="d", name=f"d{bi}")
                    sq = bwork.tile([128, B * T], f16, tag="sq", name=f"sq{bi}")
                    outb = bwork.tile([128, 4], f32, tag="outb",
                                      name=f"outb{bi}")
                    nc.gpsimd.memset(outb[:], 0.0)
                    nc.vector.tensor_sub(dt_[:], pt[:, 0:B * T],
                                         pt[:, B * T:PT_W])
                    nc.scalar.activation(sq[:], dt_[:], ACT.Square)
                    nc.scalar.activation(dt_[:], sq[:], ACT.Ln,
                                         accum_out=outb[:, 0:1])
                    nc.sync.dma_start(outb_d[r0:r0 + 128, :], outb[:])

            # seg chain results -> vrow -> out (after last seg matmul)
            for i, (c0, wdt) in enumerate(segch):
                nc.scalar.activation(vrow[:, c0:c0 + wdt], ps_seg[i][:], ACT.Copy)
            nc.sync.dma_start(outv_d[:, :], vrow[:])

    nc.compile()
    return nc


def _get_nc(mu_t, md_t, gpc):
    key = (mu_t, md_t, gpc)
    if key not in _CACHE:
        _CACHE[key] = _build_nc(dict(mu_t), dict(md_t), gpc)
    return _CACHE[key]


def _plan(vals):
    """32-aligned identical per-core group layout.
    Returns (m: value->rows_per_core (aligned), order: sorted g indices)."""
    order = np.argsort(vals, kind="stable")
    uniq, counts = np.unique(vals, return_counts=True)
    m = {}
    for v, n in zip(uniq.tolist(), counts.tolist()):
        mv = -(-n // M)            # ceil
        m[int(v)] = -(-mv // 32) * 32
    return m, order, uniq.tolist(), counts.tolist()


def _positions(m, gpc):
    """Choose a group ordering minimizing legal-piece splits.
    Returns pos: v -> starting row.  Deterministic (shared by host prep
    and program builder)."""
    vs = [v for v in sorted(m) if m[v] > 0]
    tiles = _tiles_of(gpc)

    def pieces_for(order):
        off = 0
        pos = {}
        for v in order:
            pos[v] = off
            off += m[v]
        n = 0
        for v in order:
            if v < 2 or v > T:
                continue
            r0, r1 = pos[v], pos[v] + m[v]
            for (t0, tn) in tiles:
                a, b = max(r0, t0), min(r1, t0 + tn)
                if a < b:
                    n += len(_legal_pieces(a - t0, b - t0))
        return n, pos

    cands = [
        list(vs),
        sorted(vs, key=lambda v: (m[v], v)),
        sorted(vs, key=lambda v: (-m[v], v)),
        sorted(vs, key=lambda v: (m[v] != 64, m[v], v)),
        sorted(vs, key=lambda v: (m[v] != 64, -m[v], v)),
    ]
    best = None
    for order in cands:
        n, pos = pieces_for(order)
        if best is None or n < best[0]:
            best = (n, pos)
    return best[1]


def _pack(m):
    """Place groups at tile offsets so each (tile, group) intersection is a
    single legal partition range.  Returns (pos: v->row, gpc).
    Groups sized {32,64,96} pack into 128-row tiles; larger groups fall
    back to sequential placement (correct, just more run pieces)."""
    sizes = {v: s for v, s in m.items() if s > 0}
    if any(s > 96 for s in sizes.values()):
        pos = {}
        off = 0
        for v in sorted(sizes):
            pos[v] = off
            off += sizes[v]
        return pos, -(-off // 128) * 128
    pos = {}
    t = 0
    slots32 = []
    slots64 = []
    for v in sorted([v for v, s in sizes.items() if s == 96]):
        pos[v] = 128 * t
        slots32.append(128 * t + 96)
        t += 1
    g64 = sorted([v for v, s in sizes.items() if s == 64])
    for i, v in enumerate(g64):
        if i % 2 == 0:
            pos[v] = 128 * t
            if i == len(g64) - 1:
                slots64.append(128 * t + 64)
            t += 1
        else:
            pos[v] = 128 * (t - 1) + 64
    for v in sorted([v for v, s in sizes.items() if s == 32]):
        if slots32:
            pos[v] = slots32.pop()
        elif slots64:
            s = slots64.pop()
            pos[v] = s
            slots32.append(s + 32)
        else:
            pos[v] = 128 * t
            slots32.append(128 * t + 32)
            slots64.append(128 * t + 64)
            t += 1
    return pos, 128 * t


def _gsel_of(m, order, uniq, counts, gpc):
    """Balanced assignment: per value, cores with the smallest running
    totals take the remainder rows, keeping per-core real counts equal."""
    gsel = -np.ones((M, gpc), dtype=np.int64)
    pos = _positions(m, gpc)
    totals = np.zeros(M, dtype=np.int64)
    start = 0
    for v, n in zip(uniq, counts):
        arr = order[start:start + n]
        start += n
        base = n // M
        rem = n - base * M
        cnt = np.full(M, base, dtype=np.int64)
        if rem:
            pick = np.argsort(totals, kind="stable")[:rem]
            cnt[pick] += 1
        a = 0
        for c in range(M):
            take = arr[a:a + cnt[c]]
            a += cnt[c]
            gsel[c, pos[v]:pos[v] + len(take)] = take
            totals[c] += cnt[c]
    return gsel


def _prep(inputs):
    import concourse.mybir as mybir
    f16 = np.float16
    f8 = mybir.dt.np(mybir.dt.float8e4)

    U = np.maximum(np.asarray(inputs["min_uptimes"], dtype=np.int64), 0)
    D = np.maximum(np.asarray(inputs["min_downtimes"], dtype=np.int64), 0)
    mu, ordU, unU, cnU = _plan(U)
    md, ordD, unD, cnD = _plan(D)
    gpc = max(sum(mu.values()), sum(md.values()))
    gpc = -(-gpc // 64) * 64
    for mm in (mu, md):
        used = sum(mm.values())
        if used < gpc:
            mm[0] = mm.get(0, 0) + (gpc - used)
    gselU = _gsel_of(mu, ordU, unU, cnU, gpc)
    gselD = _gsel_of(md, ordD, unD, cnD, gpc)

    s_full = np.asarray(inputs["thermal_on_rounded"], dtype=np.float32)
    ic = np.asarray(inputs["initial_commitment"], dtype=np.float32)
    p_full = np.asarray(inputs["thermal_on"], dtype=np.float32)
    t_full = np.asarray(inputs["tgt_thermal_commitment"], dtype=np.float32)
    sp_full = np.asarray(inputs["seg_prod"], dtype=np.float32)
    pg_full = np.asarray(inputs["profiled_generation"], dtype=np.float32)
    cr_full = np.asarray(inputs["charge_rate"], dtype=np.float32)
    dr_full = np.asarray(inputs["discharge_rate"], dtype=np.float32)
    curt = np.asarray(inputs["curtailment"], dtype=np.float32)
    chp = np.asarray(inputs["is_charging"], dtype=np.float32)
    cht = np.asarray(inputs["tgt_is_charging"], dtype=np.float32)
    dsp = np.asarray(inputs["is_discharging"], dtype=np.float32)
    dst = np.asarray(inputs["tgt_is_discharging"], dtype=np.float32)

    def spad_for(gsel):
        out = np.zeros((gpc, B, BL), dtype=np.float32)
        real = gsel >= 0
        gs = gsel[real]
        out[real, :, 0] = ic[:, gs].T
        out[real, :, 1:1 + T] = s_full[:, gs, :].transpose(1, 0, 2)
        return out.reshape(gpc, WID)

    in_maps = []
    ranks = []   # per-core dense generator order (for seg cols + bce rows)
    for c in range(M):
        gU, gD = gselU[c], gselD[c]
        s2 = np.concatenate([spad_for(gU), spad_for(gD)], axis=1)

        gs = gU[gU >= 0]         # dense rank order, len == GC
        assert len(gs) == GC
        ranks.append(gs)

        ptm = np.empty((BCR, PT_W), dtype=np.float32)
        ptm[:, 0:B * T] = 0.5
        ptm[:, B * T:] = -0.5
        ptm[:GC, 0:B * T] = p_full[:, gs, :].transpose(1, 0, 2).reshape(GC, B * T)
        ptm[:GC, B * T:] = (1.0 - t_full[:, gs, :]).transpose(1, 0, 2).reshape(GC, B * T)

        seg = sp_full[:, gs, :, :].transpose(0, 2, 1, 3).reshape(NBT, SEG_W)

        bsl = slice(BS * c, BS * (c + 1))
        pgr = np.zeros((2 * T, P + 2 * S + 1), dtype=np.float32)
        pgr[:, 0:P] = pg_full[bsl].transpose(0, 2, 1).reshape(2 * T, P)
        pgr[:, P:P + S] = cr_full[bsl].transpose(0, 2, 1).reshape(2 * T, S)
        pgr[:, P + S:P + 2 * S] = dr_full[bsl].transpose(0, 2, 1).reshape(2 * T, S)
        pgr[:, P + 2 * S] = curt[bsl].reshape(2 * T)

        w = BS * T
        sto = np.empty((S, 4 * w), dtype=np.float32)
        sto[:, 0:w] = chp[bsl].transpose(1, 0, 2).reshape(S, w)
        sto[:, w:2 * w] = (1.0 - cht[bsl]).transpose(1, 0, 2).reshape(S, w)
        sto[:, 2 * w:3 * w] = dsp[bsl].transpose(1, 0, 2).reshape(S, w)
        sto[:, 3 * w:4 * w] = (1.0 - dst[bsl]).transpose(1, 0, 2).reshape(S, w)

        in_maps.append({
            "s2": s2.astype(f16),
            "pt": ptm.astype(f16),
            "seg": np.ascontiguousarray(seg).astype(f8),
            "pgr": pgr.astype(f16),
            "sto": sto.astype(f16),
        })
    meta = (mu, md, gselU, gselD, gpc, ranks)
    return in_maps, meta


def _combine(outs, inputs, meta):
    mu, md, gselU, gselD, gpc, ranks = meta
    U_all = np.maximum(np.asarray(inputs["min_uptimes"]).astype(np.int64), 0)
    D_all = np.maximum(np.asarray(inputs["min_downtimes"]).astype(np.int64), 0)
    stat = np.asarray(inputs["initial_status"]).astype(np.int64)
    ic = np.asarray(inputs["initial_commitment"], dtype=np.float64)
    suc = np.asarray(inputs["start_up_costs"], dtype=np.float64)
    segc = np.asarray(inputs["segment_cost"], dtype=np.float64)[:, 0, :]
    puc = np.asarray(inputs["profiled_units_cost"], dtype=np.float64)
    ccost = np.asarray(inputs["charge_costs"], dtype=np.float64)
    dcost = np.asarray(inputs["discharge_costs"], dtype=np.float64)
    init_sum1 = ic[1:, :].sum(axis=0)

    viol = 0.0
    ed = 0.0
    bce_th = 0.0
    bce_sto = 0.0
    PFB_all = np.zeros((G, 9))

    for c in range(M):
        og = np.asarray(outs[c]["out_g"], dtype=np.float64)
        ob = np.asarray(outs[c]["out_b"], dtype=np.float64)
        ov = np.asarray(outs[c]["out_v"], dtype=np.float64)[0]
        os_ = np.asarray(outs[c]["out_s"], dtype=np.float64)

        gU, gD = gselU[c], gselD[c]
        rU = gU >= 0
        gs = gU[rU]
        Ug = U_all[gs]

        swon = -og[rU, C_SWON]
        tail = -og[rU, C_TLU]
        S0 = swon - tail
        V_U = -og[rU, C_VU]
        up = np.where((Ug >= 1) & (Ug <= T), (Ug - 1) * S0 - V_U, 0.0)
        viol += up.sum()

        rD = gD >= 0
        gsd = gD[rD]
        Dg = D_all[gsd]
        V_D = -og[rD, C_VD]
        viol += np.where((Dg >= 1) & (Dg <= T), V_D, 0.0).sum()

        PFB_all[gs, 1:9] = (og[rU, C_A0 + 1:C_A0 + 9]
                            - og[rU, C_A0:C_A0 + 1])

        ed += (suc[gs] * swon).sum()
        segv = ov[0:SEG_W].reshape(GC, K)
        ed += (segc[ranks[c]] * segv).sum()
        ed += (puc * ov[SEG_W:SEG_W + P]).sum()
        ed += (ccost * ov[SEG_W + P:SEG_W + P + S]).sum()
        ed += (dcost * ov[SEG_W + P + S:SEG_W + P + 2 * S]).sum()
        ed += POWER_BALANCE_PENALTY * ov[SEG_W + P + 2 * S]

        bce_th += ob[:, 0].sum()
        bce_sto += os_[:, 0].sum()

    rem_up = np.maximum(U_all - np.maximum(stat, 0), 0)
    rem_dn = np.maximum(D_all - np.maximum(-stat, 0), 0)
    g_idx = np.arange(G)
    viol += (B * rem_up - PFB_all[g_idx, rem_up]).sum()
    viol += PFB_all[g_idx, rem_dn].sum()

    sup = -(0.5 * bce_th) / float(B * G * T) - (0.5 * bce_sto) / float(B * S * T)
    total = ed + sup + VIOLATIONS_PENALTY * viol
    return np.float32(total)


def kernel(**inputs):
    from concourse.bass_utils import run_bass_kernel_spmd

    in_maps, meta = _prep(inputs)
    mu, md, gpc = meta[0], meta[1], meta[4]
    nc = _get_nc(tuple(sorted(mu.items())), tuple(sorted(md.items())), gpc)
    res = run_bass_kernel_spmd(nc, in_maps, core_ids=list(range(M)))
    return _combine(res.results, inputs, meta)
